# revision 1
# baseline (speedup 1.0000x reference)
"""Causal-self-attention (non-causal SDPA + RoPE) Bass kernel for 8 Trainium2 cores.

Sharding: head-parallel. 16 heads / 8 cores = 2 heads per core, all 4 batches.
Each core computes QKV projections for its 2 heads (tensor-parallel split of
Wqkv rows), RoPE, full attention for its 8 (batch, head) units, and a partial
output projection against its 128-column slice of Wout. The 8 partial outputs
are summed on the host (the all-reduce of the tensor-parallel out-proj).

Layouts on device (per core):
  xt      [1024, 8192]  X^T, f32r; column m = b*2048 + t (b-major)
  qt/kt   [128, 8192]   Q^T/K^T after RoPE; rows = 2 heads x 64 dims
  v       [128, 16*130] per batch: 16 s-tiles of [128s, 65+65] = [V_A|1 | V_B|1], bf16
  scores  S^T[s, t] via row-packed K=64 matmuls (2 heads concurrent on the PE)
  exp     ScalarE, scale=0.125 fused, no max-subtraction (scores ~ N(0,1))
  PV      attn^T[d, t] = [V|1].T @ E^T ; M=65 -> row 64 = softmax denominator
  outproj bf16: Wout slice^T as lhsT, scaled attn^T as rhs -> out^T [f, m]

Emission is software-pipelined per (batch, t-window): attention(b, tc) is
followed by one quarter of proj(b+1) and one quarter of outproj(b-1), so the
PE fills ScalarE-bound attention windows with projection work and ScalarE
never starves at batch boundaries.
"""

import numpy as np

EMBED = 1024
NUM_HEADS = 16
HEAD_DIM = 64
T = 2048
B = 4
NCORES = 8
M = T * B  # 8192
ROPE_BASE = 10000.0


def _build_program():
    import concourse.bass as bass  # noqa: F401
    import concourse.mybir as mybir
    import concourse.tile as tile
    from concourse import bacc

    dt = mybir.dt
    F32, F32R, BF16 = dt.float32, dt.float32r, dt.bfloat16
    AF = mybir.ActivationFunctionType

    nc = bacc.Bacc("TRN2", target_bir_lowering=False, debug=False,
                   num_devices=NCORES)

    xt = nc.dram_tensor("xt", [EMBED, M], F32R, kind="ExternalInput")
    wq = nc.dram_tensor("wq", [EMBED, 128], F32R, kind="ExternalInput")
    wk = nc.dram_tensor("wk", [EMBED, 128], F32R, kind="ExternalInput")
    wv = nc.dram_tensor("wv", [EMBED, 128], F32R, kind="ExternalInput")
    wo = nc.dram_tensor("wo", [128, EMBED], BF16, kind="ExternalInput")
    cosd = nc.dram_tensor("cosd", [128, T], F32, kind="ExternalInput")
    sind = nc.dram_tensor("sind", [128, T], F32, kind="ExternalInput")
    identd = nc.dram_tensor("identd", [128, 128], F32R, kind="ExternalInput")
    onesd = nc.dram_tensor("onesd", [1, 128], F32R, kind="ExternalInput")
    out = nc.dram_tensor("out", [EMBED, M], F32, kind="ExternalOutput")

    ST = 16            # s-tiles per batch (2048/128)
    VST = 130          # per-s-tile V columns: [V_A | 1 | V_B | 1]

    with tile.TileContext(nc) as tc:
        with (
            tc.tile_pool(name="const", bufs=1) as cpool,
            tc.tile_pool(name="xt", bufs=16) as xpool,
            tc.tile_pool(name="big", bufs=1) as big,
            tc.tile_pool(name="vt", bufs=2) as vtpool,
            tc.tile_pool(name="rt", bufs=2) as rtpool,
            tc.tile_pool(name="et", bufs=3) as epool,
            tc.tile_pool(name="sc", bufs=2) as scpool,
            tc.tile_pool(name="ob", bufs=3) as opool,
            tc.tile_pool(name="pp", bufs=2, space="PSUM") as pp,
            tc.tile_pool(name="ps", bufs=2, space="PSUM") as ps,
            tc.tile_pool(name="pa", bufs=2, space="PSUM") as pa,
        ):
            # ---- constants ----
            wq_sb = cpool.tile([128, 1024], F32R, tag="wq")
            wk_sb = cpool.tile([128, 1024], F32R, tag="wk")
            wv_sb = cpool.tile([128, 1024], F32R, tag="wv")
            for e in range(8):
                nc.sync.dma_start(wq_sb[:, e * 128:(e + 1) * 128],
                                  wq[e * 128:(e + 1) * 128, :])
                nc.sync.dma_start(wk_sb[:, e * 128:(e + 1) * 128],
                                  wk[e * 128:(e + 1) * 128, :])
                nc.sync.dma_start(wv_sb[:, e * 128:(e + 1) * 128],
                                  wv[e * 128:(e + 1) * 128, :])

            cos_sb = cpool.tile([128, T], F32, tag="cos")
            sin_sb = cpool.tile([128, T], F32, tag="sin")
            ident = cpool.tile([128, 128], F32R, tag="ident")
            ones1 = cpool.tile([1, 128], F32R, tag="ones")
            wo_sb = cpool.tile([128, 1024], BF16, tag="wo")

            def load_tables():
                nc.sync.dma_start(cos_sb[:], cosd[:])
                nc.sync.dma_start(sin_sb[:], sind[:])
                nc.sync.dma_start(ident[:], identd[:])
                nc.sync.dma_start(ones1[:], onesd[:])
                nc.sync.dma_start(wo_sb[:], wo[:])

            warm = cpool.tile([1, 128], F32, tag="warm")
            nc.scalar.activation(warm[:], ones1[:].bitcast(F32),
                                 AF.Exp, scale=0.0)
            qt_sb = big.tile([128, M], F32R, tag="qt")
            kt_sb = big.tile([128, M], F32R, tag="kt")
            v_sb = [big.tile([128, ST * VST], BF16, tag=f"v{b}", name=f"v_sb{b}")
                    for b in range(B)]
            attnS = {(b, g): rtpool.tile([128, 512], BF16, tag="attnS",
                                          name=f"attnS{b}_{g}")
                     for b in range(B) for g in range(4)}

            def load_x_half(h, mc):
                """Load 8 e-chunk tiles of X^T for 512 m-cols at h*1024+mc*512."""
                c0 = h * 1024 + mc * 512
                xts = [xpool.tile([128, 512], F32R, tag="xt",
                                  name=f"xt{h}_{mc}_{e}")
                       for e in range(8)]
                for e in range(8):
                    nc.sync.dma_start(xts[e][:],
                                      xt[e * 128:(e + 1) * 128, c0:c0 + 512])
                return xts

            def rope(p, dst, col0):
                """dst = cos*p + sin_eff*shift32(p), all [128, 512]."""
                pr = rtpool.tile([128, 512], F32, tag="proj_sb")
                nc.any.tensor_copy(pr[:], p[:])
                prs = rtpool.tile([128, 512], F32, tag="ropeshuf")
                for (ob, ib) in ((0, 32), (32, 0), (64, 96), (96, 64)):
                    nc.sync.dma_start(prs[ob:ob + 32, :], pr[ib:ib + 32, :])
                t2 = rtpool.tile([128, 512], F32, tag="ropetmp")
                tc0 = col0 % T
                nc.vector.tensor_mul(t2[:], prs[:], sin_sb[:, tc0:tc0 + 512])
                nc.vector.tensor_mul(dst, pr[:], cos_sb[:, tc0:tc0 + 512])
                nc.vector.tensor_add(dst, dst, t2[:])

            def proj_chunk(xts, h, mc):
                """Q/K/V projections + rope + V transpose for 512 m-columns."""
                b = h // 2
                col0 = h * 1024 + mc * 512
                for w_sb, dst in ((wq_sb, qt_sb), (wk_sb, kt_sb)):
                    p = pp.tile([128, 512], F32, tag="pp")
                    for e in range(8):
                        nc.tensor.matmul(
                            p[:], w_sb[:, e * 128:(e + 1) * 128], xts[e][:],
                            start=(e == 0), stop=(e == 7))
                    rope(p, dst[:, col0:col0 + 512], col0)
                p = pp.tile([128, 512], F32, tag="pp")
                for e in range(8):
                    nc.tensor.matmul(
                        p[:], wv_sb[:, e * 128:(e + 1) * 128], xts[e][:],
                        start=(e == 0), stop=(e == 7))
                vt = vtpool.tile([128, 512], F32R, tag="vt")
                nc.any.tensor_copy(vt[:], p[:])
                for k in range(4):
                    stt = (col0 % T) // 128 + k  # s-tile index 0..15
                    pt = pp.tile([128, 128], F32R, tag="pp")
                    nc.tensor.transpose(pt[:], vt[:, k * 128:(k + 1) * 128],
                                        ident[:])
                    # single strided copy: cols 0-63 -> +0, 64-127 -> +65
                    dstv = v_sb[b][:, stt * VST:stt * VST + 130]
                    nc.vector.tensor_copy(
                        dstv.rearrange("p (h c) -> p h c", c=65)[:, :, 0:64],
                        pt.rearrange("p (h c) -> p h c", c=64))

            def set_v_ones(b):
                nc.vector.memset(
                    v_sb[b].rearrange("p (s c) -> p s c", c=VST)[:, :, 64:65], 1.0)
                nc.vector.memset(
                    v_sb[b].rearrange("p (s c) -> p s c", c=VST)[:, :, 129:130], 1.0)

            def attention_tc(b, tcg):
                c0 = b * T + tcg * 512
                att_A = pa.tile([128, 512], F32, tag="pa")
                att_B = pa.tile([128, 512], F32, tag="pa")
                for st in range(ST):
                    s0 = b * T + st * 128
                    sab = ps.tile([128, 1024], F32, tag="sab")
                    nc.tensor.matmul(sab[:, 0:512],
                                     kt_sb[0:64, s0:s0 + 128],
                                     qt_sb[0:64, c0:c0 + 512],
                                     start=True, stop=True)
                    nc.tensor.matmul(sab[:, 512:1024],
                                     kt_sb[64:128, s0:s0 + 128],
                                     qt_sb[64:128, c0:c0 + 512],
                                     start=True, stop=True)
                    e_t = epool.tile([128, 1024], BF16, tag="et")
                    nc.scalar.activation(e_t[:], sab[:], AF.Exp, scale=0.125)
                    nc.tensor.matmul(att_A[0:65, :],
                                     v_sb[b][:, st * VST:st * VST + 65],
                                     e_t[:, 0:512],
                                     start=(st == 0), stop=(st == ST - 1))
                    nc.tensor.matmul(att_B[0:65, :],
                                     v_sb[b][:, st * VST + 65:st * VST + 130],
                                     e_t[:, 512:1024],
                                     start=(st == 0), stop=(st == ST - 1))
                # denominators -> reciprocal -> broadcast -> scale
                rec = scpool.tile([1, 1024], F32R, tag="rec")
                with nc.allow_low_precision(reason="softmax denom recip"):
                    nc.vector.reciprocal(rec[:, 0:512], att_A[64:65, :])
                    nc.vector.reciprocal(rec[:, 512:1024], att_B[64:65, :])
                for att, half in ((att_A, 0), (att_B, 1)):
                    bcp = pp.tile([128, 512], F32, tag="pp")
                    nc.tensor.matmul(bcp[:], ones1[:],
                                     rec[:, half * 512:(half + 1) * 512],
                                     start=True, stop=True)
                    bcs = scpool.tile([64, 512], F32, tag="bcs")
                    nc.any.tensor_copy(bcs[:], bcp[0:64, :])
                    nc.vector.tensor_mul(
                        attnS[(b, tcg)][half * 64:(half + 1) * 64, :],
                        att[0:64, :], bcs[:])

            def outproj_q(b, tcg, evict_engine=None):
                for ft in range(8):
                    po = pp.tile([128, 512], F32, tag="pp")
                    nc.tensor.matmul(po[:],
                                     wo_sb[:, ft * 128:(ft + 1) * 128],
                                     attnS[(b, tcg)][:],
                                     start=True, stop=True)
                    o_sb = opool.tile([128, 512], F32, tag="ob")
                    if evict_engine is None:
                        nc.vector.tensor_copy(o_sb[:], po[:])
                    else:
                        evict_engine.activation(
                            o_sb[:], po[:],
                            mybir.ActivationFunctionType.Copy)
                    nc.sync.dma_start(
                        out[ft * 128:(ft + 1) * 128,
                            b * T + tcg * 512:b * T + (tcg + 1) * 512],
                        o_sb[:])

            # ---- software-pipelined emission ----
            for b in range(B):
                set_v_ones(b)
            first = load_x_half(0, 0)
            load_tables()
            proj_chunk(first, 0, 0)
            for g in range(1, 4):
                xts = load_x_half(g // 2, g % 2)
                proj_chunk(xts, g // 2, g % 2)
            prev = None
            for b in range(B):
                for tcg in range(4):
                    attention_tc(b, tcg)
                    if b + 1 < B:
                        h, mc = 2 * (b + 1) + tcg // 2, tcg % 2
                        xts = load_x_half(h, mc)
                        proj_chunk(xts, h, mc)
                    if prev is not None:
                        outproj_q(*prev)
                    prev = (b, tcg)
            outproj_q(*prev, evict_engine=nc.scalar)

    nc.compile()
    return nc


def _host_prep(query, Wqkv, Wout):
    import ml_dtypes

    q32 = np.asarray(query, dtype=np.float32)
    # [T, B, E] -> [E, B, T] -> [E, B*T]  (column = b*T + t)
    xt = np.ascontiguousarray(q32.transpose(2, 1, 0).reshape(EMBED, M))

    # rope tables, fp16-rounded like the reference
    theta = np.power(ROPE_BASE,
                     -np.arange(0, HEAD_DIM, 2, dtype=np.float32) / HEAD_DIM)
    m_th = np.arange(T, dtype=np.float32)[:, None] * theta[None, :]
    m_th = np.concatenate([m_th, m_th], axis=-1)          # [T, 64]
    cos = np.cos(m_th).astype(np.float16).astype(np.float32)
    sin = np.sin(m_th).astype(np.float16).astype(np.float32)
    cosT = cos.T                                          # [64, T]
    sin_eff = sin.T.copy()
    sin_eff[0:32] = -sin_eff[0:32]
    cos128 = np.ascontiguousarray(np.concatenate([cosT, cosT], axis=0))
    sin128 = np.ascontiguousarray(np.concatenate([sin_eff, sin_eff], axis=0))

    W = np.asarray(Wqkv, dtype=np.float32)
    Wo = np.asarray(Wout, dtype=np.float32)
    in_maps = []
    for c in range(NCORES):
        sl = slice(c * 128, (c + 1) * 128)
        in_maps.append({
            "xt": xt,
            "wq": np.ascontiguousarray(W[sl, :].T),
            "wk": np.ascontiguousarray(W[EMBED:][sl, :].T),
            "wv": np.ascontiguousarray(W[2 * EMBED:][sl, :].T),
            "wo": np.ascontiguousarray(Wo[:, sl].T).astype(ml_dtypes.bfloat16),
            "cosd": cos128,
            "sind": sin128,
            "identd": np.eye(128, dtype=np.float32),
            "onesd": np.ones((1, 128), dtype=np.float32),
        })
    return in_maps


def kernel(query, Wqkv, Wout):
    from concourse.bass_utils import run_bass_kernel_spmd

    nc = _build_program()
    in_maps = _host_prep(query, Wqkv, Wout)
    res = run_bass_kernel_spmd(nc, in_maps, core_ids=list(range(NCORES)))
    acc = np.zeros((EMBED, M), dtype=np.float32)
    for r in res.results:
        acc += r["out"]
    # out^T [E, b*T+t] -> [B, T, E] -> [T, B, E]
    full = acc.T.reshape(B, T, EMBED).transpose(1, 0, 2)
    return np.ascontiguousarray(full)



# revision 11
# speedup vs baseline: 1.0512x; 1.0512x over previous
"""Causal-self-attention (non-causal SDPA + RoPE) Bass kernel for 8 Trainium2 cores.

Sharding: head-parallel. 16 heads / 8 cores = 2 heads per core, all 4 batches.
Each core computes QKV projections for its 2 heads (tensor-parallel split of
Wqkv rows), RoPE, full attention for its 8 (batch, head) units, and a partial
output projection against its 128-column slice of Wout. The 8 partial outputs
(bf16) are summed on the host (the all-reduce of the tensor-parallel out-proj).

Key structure (vs the straightforward version):
  - PV matmul is transposed: stationary = exp-tile [s,128t] chunk, moving =
    V [s,64d] -> psum attn^T [t, d] at 64 rows/matmul (half the PE rows of
    moving-E PV). Softmax denominators come from 1-row ones-moving matmuls
    into the same-partition psum; normalization is then a per-partition
    tensor_scalar_mul on DVE (no broadcast matmuls).
  - attn^T is normalized to bf16, PE-transposed back to [d, t] for the
    out-projection (moving = attnS bf16).
  - Emission is a per-s-tile software pipeline: window(st) = scores(st),
    exp(st) on ScalarE, PV(st-1), plus a PE-cycle-weighted slice of filler
    (next batch's QKV proj / previous tcg's outproj) so the PE never idles
    and stays at max p-state.
  - RoPE multiplies/adds run on the Pool (GPSIMD) engine; DVE keeps the
    PSUM evictions. Output DMA is bf16.
"""

import numpy as np

EMBED = 1024
NUM_HEADS = 16
HEAD_DIM = 64
T = 2048
B = 4
NCORES = 8
M = T * B  # 8192
ROPE_BASE = 10000.0


def _build_program():
    import concourse.bass as bass  # noqa: F401
    import concourse.mybir as mybir
    import concourse.tile as tile
    from concourse import bacc

    dt = mybir.dt
    F32, F32R, BF16 = dt.float32, dt.float32r, dt.bfloat16
    AF = mybir.ActivationFunctionType

    nc = bacc.Bacc("TRN2", target_bir_lowering=False, debug=False,
                   num_devices=NCORES)

    xt = nc.dram_tensor("xt", [EMBED, M], F32R, kind="ExternalInput")
    wq = nc.dram_tensor("wq", [EMBED, 128], F32R, kind="ExternalInput")
    wk = nc.dram_tensor("wk", [EMBED, 128], F32R, kind="ExternalInput")
    wv = nc.dram_tensor("wv", [EMBED, 128], F32R, kind="ExternalInput")
    wo = nc.dram_tensor("wo", [128, EMBED], BF16, kind="ExternalInput")
    cosd = nc.dram_tensor("cosd", [128, T], F32, kind="ExternalInput")
    sind = nc.dram_tensor("sind", [128, T], F32, kind="ExternalInput")
    identd = nc.dram_tensor("identd", [128, 128], BF16, kind="ExternalInput")
    out = nc.dram_tensor("out", [EMBED, M], BF16, kind="ExternalOutput")

    ST = 16            # s-tiles per batch (2048/128)

    with tile.TileContext(nc) as tc:
        with (
            tc.tile_pool(name="const", bufs=1) as cpool,
            tc.tile_pool(name="xt", bufs=24) as xpool,
            tc.tile_pool(name="big", bufs=1) as big,
            tc.tile_pool(name="vt", bufs=2) as vtpool,
            tc.tile_pool(name="rt", bufs=3) as rtpool,
            tc.tile_pool(name="et", bufs=3) as epool,
            tc.tile_pool(name="asr", bufs=4) as asrpool,
            tc.tile_pool(name="asS", bufs=2) as aspool,
            tc.tile_pool(name="rc", bufs=2) as rcpool,
            tc.tile_pool(name="ob", bufs=4) as opool,
            tc.tile_pool(name="ps", bufs=2, space="PSUM") as ps,
            tc.tile_pool(name="pv", bufs=1, space="PSUM") as pvpool,
            tc.tile_pool(name="pm", bufs=1, space="PSUM") as pmpool,
            tc.tile_pool(name="pp", bufs=2, space="PSUM") as pp,
        ):
            # ---- constants ----
            wq_sb = cpool.tile([128, 1024], F32R, tag="wq")
            wk_sb = cpool.tile([128, 1024], F32R, tag="wk")
            wv_sb = cpool.tile([128, 1024], F32R, tag="wv")
            cos_sb = cpool.tile([128, T], F32, tag="cos")
            sin_sb = cpool.tile([128, T], F32, tag="sin")
            identb = cpool.tile([128, 128], BF16, tag="identb")
            ones_bf = cpool.tile([128, 1], BF16, tag="ones_bf")
            wo_sb = cpool.tile([128, 1024], BF16, tag="wo")

            def load_consts():
                for e in range(8):
                    nc.sync.dma_start(wq_sb[:, e * 128:(e + 1) * 128],
                                      wq[e * 128:(e + 1) * 128, :])
                    nc.sync.dma_start(wk_sb[:, e * 128:(e + 1) * 128],
                                      wk[e * 128:(e + 1) * 128, :])
                    nc.sync.dma_start(wv_sb[:, e * 128:(e + 1) * 128],
                                      wv[e * 128:(e + 1) * 128, :])
                nc.sync.dma_start(identb[:], identd[:])
                nc.sync.dma_start(wo_sb[:], wo[:])
                nc.sync.dma_start(cos_sb[:], cosd[:])
                nc.sync.dma_start(sin_sb[:], sind[:])

            warm = cpool.tile([1, 64], F32, tag="warm")
            qt_sb = big.tile([128, M], F32R, tag="qt")
            kt_sb = big.tile([128, M], F32R, tag="kt")
            # V per batch: [s, st*128 + head*64 + d] bf16
            v_sb = [big.tile([128, ST * 128], BF16, tag=f"v{b}", name=f"v_sb{b}")
                    for b in range(B)]

            # psum bank maps
            # pv: one bank, 8 accumulation groups of [128t, 64d] per tcg
            pv = pvpool.tile([128, 512], F32, tag="pv")
            # pm: one bank: denoms [128,8] f32 | 2x V-transpose [128,128] bf16
            #     | 2x attnS-transpose [128,128] bf16
            pm = pmpool.tile([128, 512], F32, tag="pm")
            pm_den = pm[:, 0:8]
            pm_pt = [pm[:, 8 + 64 * i:8 + 64 * (i + 1)].bitcast(BF16)
                     for i in range(2)]
            pm_tr = [pm[:, 136 + 64 * i:136 + 64 * (i + 1)].bitcast(BF16)
                     for i in range(2)]

            def load_x_half(h, mc):
                """Issue DMAs for 8 e-chunk tiles of X^T (512 m-cols)."""
                c0 = h * 1024 + mc * 512
                xts = [xpool.tile([128, 512], F32R, tag="xt",
                                  name=f"xt{h}_{mc}_{e}")
                       for e in range(8)]
                for e in range(8):
                    nc.sync.dma_start(xts[e][:],
                                      xt[e * 128:(e + 1) * 128, c0:c0 + 512])
                return xts

            def rope(p, dst, col0):
                """dst = cos*p + sin_eff*shift32(p), all [128, 512].

                DVE: psum->sbuf copy + cos-mul; Pool: sin-mul + add."""
                pr = rtpool.tile([128, 512], F32, tag="proj_sb")
                nc.vector.tensor_copy(pr[:], p[:])
                prs = rtpool.tile([128, 512], F32, tag="ropeshuf")
                for (ob, ib) in ((0, 32), (32, 0), (64, 96), (96, 64)):
                    nc.sync.dma_start(prs[ob:ob + 32, :], pr[ib:ib + 32, :])
                tc0 = col0 % T
                u = rtpool.tile([128, 512], F32, tag="ropetmp")
                nc.vector.tensor_mul(u[:], pr[:], cos_sb[:, tc0:tc0 + 512])
                t2 = rtpool.tile([128, 512], F32, tag="ropetmp2")
                nc.gpsimd.tensor_mul(t2[:], prs[:], sin_sb[:, tc0:tc0 + 512])
                nc.gpsimd.tensor_add(dst, u[:], t2[:])

            # ---------- filler thunk machinery ----------
            # Each thunk: (pe_cycles, fn). fn() emits instructions.
            # Two priorities: "fast" (attnS transposes + outproj — must drain
            # within the next tcg so their tile slots recycle) and "slow"
            # (next batch's projections — drain across the whole batch).
            fast_q, slow_q = [], []
            fast_pos, slow_pos = [0], [0]
            fill_done = [0.0]
            fill_target = [0.0]

            def _pop_one():
                if fast_pos[0] < len(fast_q):
                    w, fn = fast_q[fast_pos[0]]
                    fast_pos[0] += 1
                elif slow_pos[0] < len(slow_q):
                    w, fn = slow_q[slow_pos[0]]
                    slow_pos[0] += 1
                else:
                    return False
                fn()
                fill_done[0] += w
                return True

            def emit_filler(budget_cycles):
                fill_target[0] += budget_cycles
                while fill_done[0] < fill_target[0]:
                    if not _pop_one():
                        break

            def drain_filler():
                while _pop_one():
                    pass

            # ---------- projection chunk (as thunks) ----------
            def proj_thunks(h, mc, xts):
                """Thunks for one 512-m-col chunk: Q, K (f32r + rope) and V
                (bf16, transposed into v_sb)."""
                b = h // 2
                col0 = h * 1024 + mc * 512
                st0 = (col0 % T) // 128
                state = {}

                def qk(which, w_sb, dst, lo, hi):
                    def fn():
                        if lo == 0:
                            state[which] = pp.tile([128, 512], F32,
                                                   tag="pp", name=f"pp_{which}_{h}_{mc}")
                        p = state[which]
                        for e in range(lo, hi):
                            nc.tensor.matmul(
                                p[:], w_sb[:, e * 128:(e + 1) * 128], xts[e][:],
                                start=(e == 0), stop=(e == 7))
                        if hi == 8:
                            rope(p, dst[:, col0:col0 + 512], col0)
                    return fn

                def vmm(lo, hi):
                    def fn():
                        if lo == 0:
                            state['v'] = pp.tile([128, 512], F32, tag="pp",
                                                 name=f"pp_v_{h}_{mc}")
                        p = state['v']
                        for e in range(lo, hi):
                            nc.tensor.matmul(
                                p[:], wv_sb[:, e * 128:(e + 1) * 128], xts[e][:],
                                start=(e == 0), stop=(e == 7))
                        if hi == 8:
                            vt = vtpool.tile([128, 512], BF16, tag="vt")
                            nc.vector.tensor_copy(vt[:], p[:])
                            state['vt'] = vt
                    return fn

                def vtr(k):
                    def fn():
                        pt = pm_pt[k % 2]
                        nc.tensor.transpose(pt, state['vt'][:, k * 128:(k + 1) * 128],
                                            identb[:])
                        nc.vector.tensor_copy(
                            v_sb[b][:, (st0 + k) * 128:(st0 + k + 1) * 128], pt)
                    return fn

                th = []
                for lo in range(0, 8, 4):
                    th.append((512 * 4, qk('q', wq_sb, qt_sb, lo, lo + 4)))
                for lo in range(0, 8, 4):
                    th.append((512 * 4, qk('k', wk_sb, kt_sb, lo, lo + 4)))
                for lo in range(0, 8, 4):
                    th.append((512 * 4, vmm(lo, lo + 4)))
                for k in range(4):
                    th.append((128, vtr(k)))
                return th

            # ---------- outproj (as thunks) ----------
            def outproj_thunks(b, tcg, attnS, last=False):
                def ft_fn(ft):
                    def fn():
                        po = pp.tile([128, 512], F32, tag="pp",
                                     name=f"po_{b}_{tcg}_{ft}")
                        nc.tensor.matmul(po[:],
                                         wo_sb[:, ft * 128:(ft + 1) * 128],
                                         attnS[:],
                                         start=True, stop=True)
                        o_sb = opool.tile([128, 512], BF16, tag="ob")
                        if last and ft >= 6:
                            nc.scalar.activation(o_sb[:], po[:], AF.Copy)
                        else:
                            nc.vector.tensor_copy(o_sb[:], po[:])
                        nc.sync.dma_start(
                            out[ft * 128:(ft + 1) * 128,
                                b * T + tcg * 512:b * T + (tcg + 1) * 512],
                            o_sb[:])
                    return fn
                return [(512, ft_fn(ft)) for ft in range(8)]

            # ---------- attention ----------
            def scores_exp(b, tcg, st):
                """scores(st) into a rotating sab buffer + exp on ScalarE.
                Returns the e_t tile."""
                c0 = b * T + tcg * 512
                s0 = b * T + st * 128
                sab = ps.tile([128, 1024], F32, tag="sab")
                nc.tensor.matmul(sab[:, 0:512],
                                 kt_sb[0:64, s0:s0 + 128],
                                 qt_sb[0:64, c0:c0 + 512],
                                 start=True, stop=True)
                nc.tensor.matmul(sab[:, 512:1024],
                                 kt_sb[64:128, s0:s0 + 128],
                                 qt_sb[64:128, c0:c0 + 512],
                                 start=True, stop=True)
                e_t = epool.tile([128, 1024], BF16, tag="et")
                nc.scalar.activation(e_t[:], sab[:], AF.Exp, scale=0.125)
                return e_t

            def pv_mms(b, st, e_t):
                """Transposed PV + denominator mms for s-tile st."""
                for tcq in range(4):
                    for hh in range(2):
                        g = tcq * 2 + hh
                        lhs = e_t[:, hh * 512 + tcq * 128:hh * 512 + (tcq + 1) * 128]
                        nc.tensor.matmul(
                            pv[:, g * 64:(g + 1) * 64], lhs,
                            v_sb[b][:, st * 128 + hh * 64:st * 128 + hh * 64 + 64],
                            start=(st == 0), stop=(st == ST - 1))
                        nc.tensor.matmul(
                            pm_den[:, g:g + 1], lhs, ones_bf[:],
                            start=(st == 0), stop=(st == ST - 1))

            def finish_tcg(b, tcg):
                """Normalize attn^T, transpose to [d, t], return attnS tile."""
                rec = rcpool.tile([128, 8], F32, tag="rec")
                with nc.allow_low_precision(reason="softmax denom recip"):
                    nc.vector.reciprocal(rec[:], pm_den[:])
                attnS = aspool.tile([128, 512], BF16, tag="attnS",
                                    name=f"attnS_{b}_{tcg}")
                asr = {}
                for tcq in range(4):
                    asr[tcq] = asrpool.tile([128, 128], BF16, tag="asr",
                                            name=f"asr_{b}_{tcg}_{tcq}")
                    for hh in range(2):
                        g = tcq * 2 + hh
                        nc.vector.tensor_scalar_mul(
                            asr[tcq][:, hh * 64:(hh + 1) * 64],
                            pv[:, g * 64:(g + 1) * 64],
                            rec[:, g:g + 1])

                def tr_fn(tcq):
                    def fn():
                        t = pm_tr[tcq % 2]
                        nc.tensor.transpose(t, asr[tcq][:], identb[:])
                        nc.vector.tensor_copy(
                            attnS[:, tcq * 128:(tcq + 1) * 128], t)
                    return fn
                th = [(128, tr_fn(tcq)) for tcq in range(4)]
                return attnS, th

            # ---------- emission ----------
            load_consts()
            nc.scalar.activation(warm[:], identb[0:1, :].bitcast(F32),
                                 AF.Exp, scale=0.0)
            nc.vector.memset(ones_bf[:], 1.0)

            # batch-0 projection runs in the open (PE otherwise idle)
            chunk_order = [(h, mc) for h in range(8) for mc in range(2)]
            xts_pending = {}
            for (h, mc) in chunk_order[:4]:
                xts_pending[(h, mc)] = load_x_half(h, mc)
            for ci, (h, mc) in enumerate(chunk_order[:4]):
                if ci + 2 < len(chunk_order) and ci >= 2:
                    nh, nmc = chunk_order[ci + 2]
                    xts_pending[(nh, nmc)] = load_x_half(nh, nmc)
                for _, fn in proj_thunks(h, mc, xts_pending.pop((h, mc))):
                    fn()

            # enqueue helper: chunk DMAs issued two chunks ahead
            next_dma = [6]

            def enqueue_proj(ci):
                h, mc = chunk_order[ci]
                if (h, mc) not in xts_pending:
                    xts_pending[(h, mc)] = load_x_half(h, mc)
                xts = xts_pending.pop((h, mc))
                th = proj_thunks(h, mc, xts)

                def prefetch():
                    if next_dma[0] < len(chunk_order):
                        nh, nmc = chunk_order[next_dma[0]]
                        xts_pending[(nh, nmc)] = load_x_half(nh, nmc)
                        next_dma[0] += 1
                w0, f0 = th[0]

                def first():
                    prefetch()
                    f0()
                slow_q.append((w0, first))
                slow_q.extend(th[1:])

            # Per-window filler budget (PE cycles). Chosen so the slow queue
            # drains each batch's projections within the preceding batch's
            # attention phase even after the fast queue takes its share.
            FILL_W = 1150.0

            prev = None          # (b, tcg, attnS)
            for b in range(B):
                if b + 1 < B:
                    for ci in range(4 * (b + 1), 4 * (b + 2)):
                        enqueue_proj(ci)
                for tcg in range(4):
                    pend = None
                    for st in range(ST):
                        e_t = scores_exp(b, tcg, st)
                        if pend is not None:
                            pv_mms(b, pend[0], pend[1])
                        pend = (st, e_t)
                        emit_filler(FILL_W + (520 if st == 0 else 0))
                    pv_mms(b, pend[0], pend[1])
                    attnS, tr_th = finish_tcg(b, tcg)
                    fast_q.extend(tr_th)
                    if prev is not None:
                        fast_q.extend(outproj_thunks(prev[0], prev[1], prev[2]))
                    prev = (b, tcg, attnS)
            drain_filler()
            for _, fn in outproj_thunks(prev[0], prev[1], prev[2], last=True):
                fn()

    nc.compile()
    return nc


def _host_prep(query, Wqkv, Wout):
    import ml_dtypes

    q32 = np.asarray(query, dtype=np.float32)
    # [T, B, E] -> [E, B, T] -> [E, B*T]  (column = b*T + t)
    xt = np.ascontiguousarray(q32.transpose(2, 1, 0).reshape(EMBED, M))

    # rope tables, fp16-rounded like the reference
    theta = np.power(ROPE_BASE,
                     -np.arange(0, HEAD_DIM, 2, dtype=np.float32) / HEAD_DIM)
    m_th = np.arange(T, dtype=np.float32)[:, None] * theta[None, :]
    m_th = np.concatenate([m_th, m_th], axis=-1)          # [T, 64]
    cos = np.cos(m_th).astype(np.float16).astype(np.float32)
    sin = np.sin(m_th).astype(np.float16).astype(np.float32)
    cosT = cos.T                                          # [64, T]
    sin_eff = sin.T.copy()
    sin_eff[0:32] = -sin_eff[0:32]
    cos128 = np.ascontiguousarray(np.concatenate([cosT, cosT], axis=0))
    sin128 = np.ascontiguousarray(np.concatenate([sin_eff, sin_eff], axis=0))

    W = np.asarray(Wqkv, dtype=np.float32)
    Wo = np.asarray(Wout, dtype=np.float32)
    in_maps = []
    for c in range(NCORES):
        sl = slice(c * 128, (c + 1) * 128)
        in_maps.append({
            "xt": xt,
            "wq": np.ascontiguousarray(W[sl, :].T),
            "wk": np.ascontiguousarray(W[EMBED:][sl, :].T),
            "wv": np.ascontiguousarray(W[2 * EMBED:][sl, :].T),
            "wo": np.ascontiguousarray(Wo[:, sl].T).astype(ml_dtypes.bfloat16),
            "cosd": cos128,
            "sind": sin128,
            "identd": np.eye(128, dtype=np.float32).astype(ml_dtypes.bfloat16),
        })
    return in_maps


def kernel(query, Wqkv, Wout):
    from concourse.bass_utils import run_bass_kernel_spmd

    nc = _build_program()
    in_maps = _host_prep(query, Wqkv, Wout)
    res = run_bass_kernel_spmd(nc, in_maps, core_ids=list(range(NCORES)))
    acc = np.zeros((EMBED, M), dtype=np.float32)
    for r in res.results:
        acc += np.asarray(r["out"], dtype=np.float32)
    # out^T [E, b*T+t] -> [B, T, E] -> [T, B, E]
    full = acc.T.reshape(B, T, EMBED).transpose(1, 0, 2)
    return np.ascontiguousarray(full)


# revision 21
# speedup vs baseline: 1.0915x; 1.0384x over previous
"""Causal-self-attention (non-causal SDPA + RoPE) Bass kernel for 8 Trainium2 cores.

Sharding: head-parallel. 16 heads / 8 cores = 2 heads per core, all 4 batches.
Each core computes QKV projections for its 2 heads (tensor-parallel split of
Wqkv rows), RoPE, full attention for its 8 (batch, head) units, and a partial
output projection against its 128-column slice of Wout. The 8 partial outputs
(bf16) are summed on the host (the all-reduce of the tensor-parallel out-proj).

Key structure (vs the straightforward version):
  - PV matmul is transposed: stationary = exp-tile [s,128t] chunk, moving =
    V [s,64d] -> psum attn^T [t, d] at 64 rows/matmul (half the PE rows of
    moving-E PV). Softmax denominators come from 1-row ones-moving matmuls
    into the same-partition psum; normalization is then a per-partition
    tensor_scalar_mul on DVE (no broadcast matmuls).
  - attn^T is normalized to bf16, PE-transposed back to [d, t] for the
    out-projection (moving = attnS bf16).
  - Emission is a per-s-tile software pipeline: window(st) = scores(st),
    exp(st) on ScalarE, PV(st-1), plus a PE-cycle-weighted slice of filler
    (next batch's QKV proj / previous tcg's outproj) so the PE never idles
    and stays at max p-state.
  - RoPE multiplies/adds run on the Pool (GPSIMD) engine; DVE keeps the
    PSUM evictions. Output DMA is bf16.
"""

import numpy as np

EMBED = 1024
NUM_HEADS = 16
HEAD_DIM = 64
T = 2048
B = 4
NCORES = 8
M = T * B  # 8192
ROPE_BASE = 10000.0


def _build_program():
    import concourse.bass as bass  # noqa: F401
    import concourse.mybir as mybir
    import concourse.tile as tile
    from concourse import bacc

    dt = mybir.dt
    F32, F32R, BF16 = dt.float32, dt.float32r, dt.bfloat16
    AF = mybir.ActivationFunctionType

    nc = bacc.Bacc("TRN2", target_bir_lowering=False, debug=False,
                   num_devices=NCORES)

    xt = nc.dram_tensor("xt", [EMBED, M], BF16, kind="ExternalInput")
    wq = nc.dram_tensor("wq", [EMBED, 128], BF16, kind="ExternalInput")
    wk = nc.dram_tensor("wk", [EMBED, 128], BF16, kind="ExternalInput")
    wv = nc.dram_tensor("wv", [EMBED, 128], BF16, kind="ExternalInput")
    wo = nc.dram_tensor("wo", [128, EMBED], BF16, kind="ExternalInput")
    cosd = nc.dram_tensor("cosd", [128, T], F32, kind="ExternalInput")
    sind = nc.dram_tensor("sind", [128, T], F32, kind="ExternalInput")
    identd = nc.dram_tensor("identd", [128, 128], BF16, kind="ExternalInput")
    out = nc.dram_tensor("out", [EMBED, M], BF16, kind="ExternalOutput")

    ST = 16            # s-tiles per batch (2048/128)

    with tile.TileContext(nc) as tc:
        with (
            tc.tile_pool(name="const", bufs=1) as cpool,
            tc.tile_pool(name="xt", bufs=24) as xpool,
            tc.tile_pool(name="big", bufs=1) as big,
            tc.tile_pool(name="vt", bufs=2) as vtpool,
            tc.tile_pool(name="rt", bufs=3) as rtpool,
            tc.tile_pool(name="et", bufs=3) as epool,
            tc.tile_pool(name="asr", bufs=4) as asrpool,
            tc.tile_pool(name="asS", bufs=8) as aspool,
            tc.tile_pool(name="rc", bufs=2) as rcpool,
            tc.tile_pool(name="ob", bufs=4) as opool,
            tc.tile_pool(name="ps", bufs=2, space="PSUM") as ps,
            tc.tile_pool(name="pv", bufs=1, space="PSUM") as pvpool,
            tc.tile_pool(name="pm", bufs=1, space="PSUM") as pmpool,
            tc.tile_pool(name="pp", bufs=2, space="PSUM") as pp,
        ):
            # ---- constants ----
            wq_sb = cpool.tile([128, 1024], BF16, tag="wq")
            wk_sb = cpool.tile([128, 1024], BF16, tag="wk")
            wv_sb = cpool.tile([128, 1024], BF16, tag="wv")
            cos_sb = cpool.tile([128, T], F32, tag="cos")
            sin_sb = cpool.tile([128, T], F32, tag="sin")
            identb = cpool.tile([128, 128], BF16, tag="identb")
            ones_bf = cpool.tile([128, 1], BF16, tag="ones_bf")
            wo_sb = cpool.tile([128, 1024], BF16, tag="wo")

            def load_weights():
                for w_sb, w_d in ((wq_sb, wq), (wk_sb, wk), (wv_sb, wv)):
                    for e in range(8):
                        nc.sync.dma_start(w_sb[:, e * 128:(e + 1) * 128],
                                          w_d[e * 128:(e + 1) * 128, :])
                nc.sync.dma_start(identb[:], identd[:])

            def load_tables():
                nc.sync.dma_start(cos_sb[:], cosd[:])
                nc.sync.dma_start(sin_sb[:], sind[:])
                nc.sync.dma_start(wo_sb[:], wo[:])

            warm = cpool.tile([1, 64], F32, tag="warm")
            qt_sb = big.tile([128, M], F32R, tag="qt")
            kt_sb = big.tile([128, M], F32R, tag="kt")
            # V per batch: [s, st*128 + head*64 + d] bf16
            v_sb = [big.tile([128, ST * 128], BF16, tag=f"v{b}", name=f"v_sb{b}")
                    for b in range(B)]

            # psum bank maps
            # pv: one bank, 8 accumulation groups of [128t, 64d] per tcg
            pv = pvpool.tile([128, 512], F32, tag="pv")
            # pm: one bank: denoms [128,8] f32 | 2x V-transpose [128,128] bf16
            #     | 2x attnS-transpose [128,128] bf16
            pm = pmpool.tile([128, 512], F32, tag="pm")
            pm_den = pm[:, 0:8]
            pm_pt = [pm[:, 8 + 64 * i:8 + 64 * (i + 1)].bitcast(BF16)
                     for i in range(2)]
            pm_tr = [pm[:, 136 + 64 * i:136 + 64 * (i + 1)].bitcast(BF16)
                     for i in range(2)]

            def load_x_half(h, mc):
                """Issue DMAs for 8 e-chunk tiles of X^T (512 m-cols)."""
                c0 = h * 1024 + mc * 512
                xts = [xpool.tile([128, 512], BF16, tag="xt",
                                  name=f"xt{h}_{mc}_{e}")
                       for e in range(8)]
                for e in range(8):
                    nc.sync.dma_start(xts[e][:],
                                      xt[e * 128:(e + 1) * 128, c0:c0 + 512])
                return xts

            def rope(p, dst, col0):
                """dst = cos*p + sin_eff*shift32(p), all [128, 512].

                DVE: psum->sbuf copy + cos-mul; Pool: sin-mul + add."""
                pr = rtpool.tile([128, 512], BF16, tag="proj_sb")
                nc.vector.tensor_copy(pr[:], p[:])
                prs = rtpool.tile([128, 512], BF16, tag="ropeshuf")
                for (ob, ib) in ((0, 32), (32, 0), (64, 96), (96, 64)):
                    nc.sync.dma_start(prs[ob:ob + 32, :], pr[ib:ib + 32, :])
                tc0 = col0 % T
                u = rtpool.tile([128, 512], F32, tag="ropetmp")
                nc.vector.tensor_mul(u[:], pr[:], cos_sb[:, tc0:tc0 + 512])
                t2 = rtpool.tile([128, 512], F32, tag="ropetmp2")
                nc.gpsimd.tensor_mul(t2[:], prs[:], sin_sb[:, tc0:tc0 + 512])
                nc.gpsimd.tensor_add(dst, u[:], t2[:])

            # ---------- filler thunk machinery ----------
            # Each thunk: (pe_cycles, fn). fn() emits instructions.
            # Two priorities: "fast" (attnS transposes + outproj — must drain
            # within the next tcg so their tile slots recycle) and "slow"
            # (next batch's projections — drain across the whole batch).
            fast_q, slow_q = [], []
            fast_pos, slow_pos = [0], [0]
            fill_done = [0.0]
            fill_target = [0.0]

            def _pop_one():
                if fast_pos[0] < len(fast_q):
                    w, fn = fast_q[fast_pos[0]]
                    fast_pos[0] += 1
                elif slow_pos[0] < len(slow_q):
                    w, fn = slow_q[slow_pos[0]]
                    slow_pos[0] += 1
                else:
                    return False
                fn()
                fill_done[0] += w
                return True

            def emit_filler(budget_cycles):
                fill_target[0] += budget_cycles
                while fill_done[0] < fill_target[0]:
                    if not _pop_one():
                        break

            def drain_filler():
                while _pop_one():
                    pass

            # ---------- projection chunk (as thunks) ----------
            def proj_thunks(h, mc, xts):
                """Thunks for one 512-m-col chunk: Q, K (f32r + rope) and V
                (bf16, transposed into v_sb)."""
                b = h // 2
                col0 = h * 1024 + mc * 512
                st0 = (col0 % T) // 128
                state = {}

                def qk(which, w_sb, dst, lo, hi):
                    def fn():
                        if lo == 0:
                            state[which] = pp.tile([128, 512], F32,
                                                   tag="pp", name=f"pp_{which}_{h}_{mc}")
                        p = state[which]
                        for e in range(lo, hi):
                            nc.tensor.matmul(
                                p[:], w_sb[:, e * 128:(e + 1) * 128], xts[e][:],
                                start=(e == 0), stop=(e == 7))
                        if hi == 8:
                            rope(p, dst[:, col0:col0 + 512], col0)
                    return fn

                def vmm(lo, hi):
                    def fn():
                        if lo == 0:
                            state['v'] = pp.tile([128, 512], F32, tag="pp",
                                                 name=f"pp_v_{h}_{mc}")
                        p = state['v']
                        for e in range(lo, hi):
                            nc.tensor.matmul(
                                p[:], wv_sb[:, e * 128:(e + 1) * 128], xts[e][:],
                                start=(e == 0), stop=(e == 7))
                        if hi == 8:
                            vt = vtpool.tile([128, 512], BF16, tag="vt")
                            nc.vector.tensor_copy(vt[:], p[:])
                            state['vt'] = vt
                    return fn

                def vtr(k):
                    def fn():
                        pt = pm_pt[k % 2]
                        nc.tensor.transpose(pt, state['vt'][:, k * 128:(k + 1) * 128],
                                            identb[:])
                        nc.vector.tensor_copy(
                            v_sb[b][:, (st0 + k) * 128:(st0 + k + 1) * 128], pt)
                    return fn

                th = []
                for lo in range(0, 8, 4):
                    th.append((512 * 4, qk('q', wq_sb, qt_sb, lo, lo + 4)))
                for lo in range(0, 8, 4):
                    th.append((512 * 4, qk('k', wk_sb, kt_sb, lo, lo + 4)))
                for lo in range(0, 8, 4):
                    th.append((512 * 4, vmm(lo, lo + 4)))
                for k in range(4):
                    th.append((128, vtr(k)))
                return th

            # ---------- outproj (as thunks) ----------
            def outproj_thunks(b, tcg, attnS, last=False):
                def ft_fn(ft):
                    def fn():
                        po = pp.tile([128, 512], F32, tag="pp",
                                     name=f"po_{b}_{tcg}_{ft}")
                        nc.tensor.matmul(po[:],
                                         wo_sb[:, ft * 128:(ft + 1) * 128],
                                         attnS[:],
                                         start=True, stop=True)
                        o_sb = opool.tile([128, 512], BF16, tag="ob")
                        if last and ft >= 6:
                            nc.scalar.activation(o_sb[:], po[:], AF.Copy)
                        else:
                            nc.vector.tensor_copy(o_sb[:], po[:])
                        nc.sync.dma_start(
                            out[ft * 128:(ft + 1) * 128,
                                b * T + tcg * 512:b * T + (tcg + 1) * 512],
                            o_sb[:])
                    return fn
                return [(512, ft_fn(ft)) for ft in range(8)]

            # ---------- attention ----------
            def scores_exp(b, tcg, st):
                """scores(st) into a rotating sab buffer + exp on ScalarE.
                Returns the e_t tile."""
                c0 = b * T + tcg * 512
                s0 = b * T + st * 128
                sab = ps.tile([128, 1024], F32, tag="sab")
                nc.tensor.matmul(sab[:, 0:512],
                                 kt_sb[0:64, s0:s0 + 128],
                                 qt_sb[0:64, c0:c0 + 512],
                                 start=True, stop=True)
                nc.tensor.matmul(sab[:, 512:1024],
                                 kt_sb[64:128, s0:s0 + 128],
                                 qt_sb[64:128, c0:c0 + 512],
                                 start=True, stop=True)
                e_t = epool.tile([128, 1024], BF16, tag="et")
                nc.scalar.activation(e_t[:], sab[:], AF.Exp, scale=0.125)
                return e_t

            def pv_mms(b, st, e_t):
                """Transposed PV + denominator mms for s-tile st."""
                for tcq in range(4):
                    for hh in range(2):
                        g = tcq * 2 + hh
                        lhs = e_t[:, hh * 512 + tcq * 128:hh * 512 + (tcq + 1) * 128]
                        nc.tensor.matmul(
                            pv[:, g * 64:(g + 1) * 64], lhs,
                            v_sb[b][:, st * 128 + hh * 64:st * 128 + hh * 64 + 64],
                            start=(st == 0), stop=(st == ST - 1))
                        nc.tensor.matmul(
                            pm_den[:, g:g + 1], lhs, ones_bf[:],
                            start=(st == 0), stop=(st == ST - 1))

            def finish_tcg(b, tcg):
                """Normalize attn^T, transpose to [d, t], return attnS tile."""
                rec = rcpool.tile([128, 8], F32, tag="rec")
                with nc.allow_low_precision(reason="softmax denom recip"):
                    nc.vector.reciprocal(rec[:], pm_den[:])
                attnS = aspool.tile([128, 512], BF16, tag="attnS",
                                    name=f"attnS_{b}_{tcg}")
                asr = {}
                for tcq in range(4):
                    asr[tcq] = asrpool.tile([128, 128], BF16, tag="asr",
                                            name=f"asr_{b}_{tcg}_{tcq}")
                    for hh in range(2):
                        g = tcq * 2 + hh
                        nc.vector.tensor_scalar_mul(
                            asr[tcq][:, hh * 64:(hh + 1) * 64],
                            pv[:, g * 64:(g + 1) * 64],
                            rec[:, g:g + 1])

                def tr_fn(tcq):
                    def fn():
                        t = pm_tr[tcq % 2]
                        nc.tensor.transpose(t, asr[tcq][:], identb[:])
                        nc.vector.tensor_copy(
                            attnS[:, tcq * 128:(tcq + 1) * 128], t)
                    return fn
                th = [(128, tr_fn(tcq)) for tcq in range(4)]
                return attnS, th

            # ---------- emission ----------
            load_weights()
            nc.scalar.activation(warm[:], identb[0:1, :].bitcast(F32),
                                 AF.Exp, scale=0.0)
            nc.vector.memset(ones_bf[:], 1.0)

            # batch-0 projection runs in the open (PE otherwise idle).
            # DMA order: weights, chunk 0-1, rope tables, chunk 2-3 — so the
            # first matmuls and the first rope are never DMA-starved.
            chunk_order = [(h, mc) for h in range(8) for mc in range(2)]
            xts_pending = {}
            for (h, mc) in chunk_order[:2]:
                xts_pending[(h, mc)] = load_x_half(h, mc)
            load_tables()
            for (h, mc) in chunk_order[2:4]:
                xts_pending[(h, mc)] = load_x_half(h, mc)
            for ci, (h, mc) in enumerate(chunk_order[:4]):
                if ci + 2 < len(chunk_order) and ci >= 2:
                    nh, nmc = chunk_order[ci + 2]
                    xts_pending[(nh, nmc)] = load_x_half(nh, nmc)
                for _, fn in proj_thunks(h, mc, xts_pending.pop((h, mc))):
                    fn()

            # enqueue helper: chunk DMAs issued two chunks ahead
            next_dma = [6]

            def enqueue_proj(ci):
                h, mc = chunk_order[ci]
                if (h, mc) not in xts_pending:
                    xts_pending[(h, mc)] = load_x_half(h, mc)
                xts = xts_pending.pop((h, mc))
                th = proj_thunks(h, mc, xts)

                def prefetch():
                    if next_dma[0] < len(chunk_order):
                        nh, nmc = chunk_order[next_dma[0]]
                        xts_pending[(nh, nmc)] = load_x_half(nh, nmc)
                        next_dma[0] += 1
                w0, f0 = th[0]

                def first():
                    prefetch()
                    f0()
                slow_q.append((w0, first))
                slow_q.extend(th[1:])

            # Per-window filler budget (PE cycles). Chosen so the slow queue
            # drains each batch's projections within the preceding batch's
            # attention phase even after the fast queue takes its share.
            FILL_W = 1000.0

            prev = None          # (b, tcg, attnS)
            for b in range(B):
                if b + 1 < B:
                    for ci in range(4 * (b + 1), 4 * (b + 2)):
                        enqueue_proj(ci)
                for tcg in range(4):
                    pend = None
                    for st in range(ST):
                        e_t = scores_exp(b, tcg, st)
                        if pend is not None:
                            pv_mms(b, pend[0], pend[1])
                        pend = (st, e_t)
                        emit_filler(FILL_W + (520 if st == 0 else 0))
                    pv_mms(b, pend[0], pend[1])
                    attnS, tr_th = finish_tcg(b, tcg)
                    fast_q.extend(tr_th)
                    if prev is not None:
                        # slow queue: outproj defers into the proj-starved
                        # final phase (attnS tiles stay alive meanwhile)
                        slow_q.extend(outproj_thunks(prev[0], prev[1], prev[2]))
                    prev = (b, tcg, attnS)
            drain_filler()
            for _, fn in outproj_thunks(prev[0], prev[1], prev[2], last=True):
                fn()

    nc.compile()
    return nc


def _host_prep(query, Wqkv, Wout):
    import ml_dtypes

    q32 = np.asarray(query, dtype=np.float32)
    # [T, B, E] -> [E, B, T] -> [E, B*T]  (column = b*T + t)
    xt = np.ascontiguousarray(q32.transpose(2, 1, 0).reshape(EMBED, M))

    # rope tables, fp16-rounded like the reference
    theta = np.power(ROPE_BASE,
                     -np.arange(0, HEAD_DIM, 2, dtype=np.float32) / HEAD_DIM)
    m_th = np.arange(T, dtype=np.float32)[:, None] * theta[None, :]
    m_th = np.concatenate([m_th, m_th], axis=-1)          # [T, 64]
    cos = np.cos(m_th).astype(np.float16).astype(np.float32)
    sin = np.sin(m_th).astype(np.float16).astype(np.float32)
    cosT = cos.T                                          # [64, T]
    sin_eff = sin.T.copy()
    sin_eff[0:32] = -sin_eff[0:32]
    cos128 = np.ascontiguousarray(np.concatenate([cosT, cosT], axis=0))
    sin128 = np.ascontiguousarray(np.concatenate([sin_eff, sin_eff], axis=0))

    W = np.asarray(Wqkv, dtype=np.float32)
    Wo = np.asarray(Wout, dtype=np.float32)
    in_maps = []
    for c in range(NCORES):
        sl = slice(c * 128, (c + 1) * 128)
        in_maps.append({
            "xt": xt.astype(ml_dtypes.bfloat16),
            "wq": np.ascontiguousarray(W[sl, :].T).astype(ml_dtypes.bfloat16),
            "wk": np.ascontiguousarray(W[EMBED:][sl, :].T).astype(
                ml_dtypes.bfloat16),
            "wv": np.ascontiguousarray(W[2 * EMBED:][sl, :].T).astype(
                ml_dtypes.bfloat16),
            "wo": np.ascontiguousarray(Wo[:, sl].T).astype(ml_dtypes.bfloat16),
            "cosd": cos128,
            "sind": sin128,
            "identd": np.eye(128, dtype=np.float32).astype(ml_dtypes.bfloat16),
        })
    return in_maps


def kernel(query, Wqkv, Wout):
    from concourse.bass_utils import run_bass_kernel_spmd

    nc = _build_program()
    in_maps = _host_prep(query, Wqkv, Wout)
    res = run_bass_kernel_spmd(nc, in_maps, core_ids=list(range(NCORES)))
    acc = np.zeros((EMBED, M), dtype=np.float32)
    for r in res.results:
        acc += np.asarray(r["out"], dtype=np.float32)
    # out^T [E, b*T+t] -> [B, T, E] -> [T, B, E]
    full = acc.T.reshape(B, T, EMBED).transpose(1, 0, 2)
    return np.ascontiguousarray(full)


# revision 32
# speedup vs baseline: 1.1900x; 1.0902x over previous
"""Causal-self-attention (non-causal SDPA + RoPE) Bass kernel for 8 Trainium2 cores.

Sharding: head-parallel. 16 heads / 8 cores = 2 heads per core, all 4 batches.
Each core computes QKV projections for its 2 heads (tensor-parallel split of
Wqkv rows), RoPE, full attention for its 8 (batch, head) units, and a partial
output projection against its 128-column slice of Wout. The 8 partial outputs
(bf16) are summed on the host (the all-reduce of the tensor-parallel out-proj).

Key structure (vs the straightforward version):
  - PV matmul is transposed: stationary = exp-tile [s,128t] chunk, moving =
    V [s,64d] -> psum attn^T [t, d] at 64 rows/matmul (half the PE rows of
    moving-E PV). Softmax denominators come from 1-row ones-moving matmuls
    into the same-partition psum; normalization is then a per-partition
    tensor_scalar_mul on DVE (no broadcast matmuls).
  - attn^T is normalized to bf16, PE-transposed back to [d, t] for the
    out-projection (moving = attnS bf16).
  - Emission is a per-s-tile software pipeline: window(st) = scores(st),
    exp(st) on ScalarE, PV(st-1), plus a PE-cycle-weighted slice of filler
    (next batch's QKV proj / previous tcg's outproj) so the PE never idles
    and stays at max p-state.
  - RoPE multiplies/adds run on the Pool (GPSIMD) engine; DVE keeps the
    PSUM evictions. Output DMA is bf16.
"""

import numpy as np

EMBED = 1024
NUM_HEADS = 16
HEAD_DIM = 64
T = 2048
B = 4
NCORES = 8
M = T * B  # 8192
ROPE_BASE = 10000.0


def _build_program():
    import concourse.bass as bass  # noqa: F401
    import concourse.mybir as mybir
    import concourse.tile as tile
    from concourse import bacc

    dt = mybir.dt
    F32, F32R, BF16 = dt.float32, dt.float32r, dt.bfloat16
    AF = mybir.ActivationFunctionType

    nc = bacc.Bacc("TRN2", target_bir_lowering=False, debug=False,
                   num_devices=NCORES)

    xt = nc.dram_tensor("xt", [EMBED, M], BF16, kind="ExternalInput")
    wq = nc.dram_tensor("wq", [EMBED, 128], BF16, kind="ExternalInput")
    wk = nc.dram_tensor("wk", [EMBED, 128], BF16, kind="ExternalInput")
    wv = nc.dram_tensor("wv", [EMBED, 128], BF16, kind="ExternalInput")
    wo = nc.dram_tensor("wo", [128, EMBED], BF16, kind="ExternalInput")
    cosd = nc.dram_tensor("cosd", [128, T], F32, kind="ExternalInput")
    sind = nc.dram_tensor("sind", [128, T], F32, kind="ExternalInput")
    identd = nc.dram_tensor("identd", [128, 128], BF16, kind="ExternalInput")
    permd = nc.dram_tensor("permd", [128, 128], BF16, kind="ExternalInput")
    out = nc.dram_tensor("out", [EMBED, M], BF16, kind="ExternalOutput")

    ST = 16            # s-tiles per batch (2048/128)

    with tile.TileContext(nc) as tc:
        with (
            tc.tile_pool(name="const", bufs=1) as cpool,
            tc.tile_pool(name="xt", bufs=3) as xpool,
            tc.tile_pool(name="big", bufs=1) as big,
            tc.tile_pool(name="vt", bufs=2) as vtpool,
            tc.tile_pool(name="rt", bufs=3) as rtpool,
            tc.tile_pool(name="et", bufs=3) as epool,
            tc.tile_pool(name="asr", bufs=4) as asrpool,
            tc.tile_pool(name="asS", bufs=8) as aspool,
            tc.tile_pool(name="rc", bufs=2) as rcpool,
            tc.tile_pool(name="ob", bufs=2) as opool,
            tc.tile_pool(name="ps", bufs=2, space="PSUM") as ps,
            tc.tile_pool(name="pv", bufs=1, space="PSUM") as pvpool,
            tc.tile_pool(name="pm", bufs=1, space="PSUM") as pmpool,
            tc.tile_pool(name="pp", bufs=2, space="PSUM") as pp,
        ):
            # ---- constants ----
            wq_sb = cpool.tile([128, 1024], BF16, tag="wq")
            wk_sb = cpool.tile([128, 1024], BF16, tag="wk")
            wv_sb = cpool.tile([128, 1024], BF16, tag="wv")
            cos_sb = cpool.tile([128, T], F32, tag="cos")
            sin_sb = cpool.tile([128, T], F32, tag="sin")
            identb = cpool.tile([128, 128], BF16, tag="identb")
            perm_sb = cpool.tile([128, 128], BF16, tag="perm")
            ones_bf = cpool.tile([128, 1], BF16, tag="ones_bf")
            wo_sb = cpool.tile([128, 1024], BF16, tag="wo")

            def load_weights():
                for w_sb, w_d in ((wq_sb, wq), (wk_sb, wk), (wv_sb, wv)):
                    nc.sync.dma_start(
                        w_sb[:].rearrange("p (e d) -> p e d", e=8),
                        w_d[:].rearrange("(e p) d -> p e d", e=8))
                nc.sync.dma_start(identb[:], identd[:])
                nc.sync.dma_start(perm_sb[:], permd[:])

            def load_tables():
                nc.sync.dma_start(cos_sb[:], cosd[:])
                nc.sync.dma_start(sin_sb[:], sind[:])
                nc.sync.dma_start(wo_sb[:], wo[:])

            warm = cpool.tile([1, 64], F32, tag="warm")
            qt_sb = big.tile([128, M], F32R, tag="qt")
            kt_sb = big.tile([128, M], F32R, tag="kt")
            # V per batch: [s, st*128 + head*64 + d] bf16
            v_sb = [big.tile([128, ST * 128], BF16, tag=f"v{b}", name=f"v_sb{b}")
                    for b in range(B)]

            # psum bank maps
            # pv: one bank, 8 accumulation groups of [128t, 64d] per tcg
            pv = pvpool.tile([128, 512], F32, tag="pv")
            # pm: one bank: denoms [128,8] f32 | 2x V-transpose [128,128] bf16
            #     | 2x attnS-transpose [128,128] bf16
            pm = pmpool.tile([128, 512], F32, tag="pm")
            pm_den = pm[:, 0:8]
            pm_pt = [pm[:, 8 + 64 * i:8 + 64 * (i + 1)].bitcast(BF16)
                     for i in range(2)]
            pm_tr = [pm[:, 136 + 64 * i:136 + 64 * (i + 1)].bitcast(BF16)
                     for i in range(2)]

            def load_x_half(h, mc):
                """One DMA for all 8 e-chunks of X^T (512 m-cols)."""
                c0 = h * 1024 + mc * 512
                xts = xpool.tile([128, 8 * 512], BF16, tag="xt",
                                 name=f"xt{h}_{mc}")
                nc.sync.dma_start(
                    xts[:].rearrange("p (e m) -> p e m", e=8),
                    xt[:, c0:c0 + 512].rearrange("(e p) m -> p e m", e=8))
                return xts

            def rope(p, dst, col0):
                """dst = cos*p + sin_eff*shift32(p), all [128, 512].

                The rotate-half partition shuffle is a PE matmul against a
                constant permutation matrix (sign lives in the sin table).
                DVE: psum->sbuf copy + sin-mul; Pool: cos-mul + add."""
                pr = rtpool.tile([128, 512], BF16, tag="proj_sb")
                nc.vector.tensor_copy(pr[:], p[:])
                prs = pp.tile([128, 512], F32, tag="pp", name=f"prs_{col0}")
                nc.tensor.matmul(prs[:], perm_sb[:], pr[:],
                                 start=True, stop=True)
                tc0 = col0 % T
                u = rtpool.tile([128, 512], F32, tag="ropetmp")
                nc.gpsimd.tensor_mul(u[:], pr[:], cos_sb[:, tc0:tc0 + 512])
                t2 = rtpool.tile([128, 512], F32, tag="ropetmp2")
                nc.vector.tensor_mul(t2[:], prs[:], sin_sb[:, tc0:tc0 + 512])
                nc.gpsimd.tensor_add(dst, u[:], t2[:])

            # ---------- filler thunk machinery ----------
            # Each thunk: (pe_cycles, fn). fn() emits instructions.
            # Two priorities: "fast" (attnS transposes + outproj — must drain
            # within the next tcg so their tile slots recycle) and "slow"
            # (next batch's projections — drain across the whole batch).
            fast_q, slow_q = [], []
            fast_pos, slow_pos = [0], [0]
            fill_done = [0.0]
            fill_target = [0.0]

            def _pop_one():
                if fast_pos[0] < len(fast_q):
                    w, fn = fast_q[fast_pos[0]]
                    fast_pos[0] += 1
                elif slow_pos[0] < len(slow_q):
                    w, fn = slow_q[slow_pos[0]]
                    slow_pos[0] += 1
                else:
                    return False
                fn()
                fill_done[0] += w
                return True

            def emit_filler(budget_cycles):
                fill_target[0] += budget_cycles
                while fill_done[0] < fill_target[0]:
                    if not _pop_one():
                        break

            def drain_filler():
                while _pop_one():
                    pass

            # ---------- projection chunk (as thunks) ----------
            def proj_thunks(h, mc, xts):
                """Thunks for one 512-m-col chunk: Q, K (f32r + rope) and V
                (bf16, transposed into v_sb)."""
                b = h // 2
                col0 = h * 1024 + mc * 512
                st0 = (col0 % T) // 128
                state = {}

                def qk(which, w_sb, dst, lo, hi):
                    def fn():
                        if lo == 0:
                            state[which] = pp.tile([128, 512], F32,
                                                   tag="pp", name=f"pp_{which}_{h}_{mc}")
                        p = state[which]
                        for e in range(lo, hi):
                            nc.tensor.matmul(
                                p[:], w_sb[:, e * 128:(e + 1) * 128],
                                xts[:, e * 512:(e + 1) * 512],
                                start=(e == 0), stop=(e == 7))
                        if hi == 8:
                            rope(p, dst[:, col0:col0 + 512], col0)
                    return fn

                def vmm(lo, hi):
                    def fn():
                        if lo == 0:
                            state['v'] = pp.tile([128, 512], F32, tag="pp",
                                                 name=f"pp_v_{h}_{mc}")
                        p = state['v']
                        for e in range(lo, hi):
                            nc.tensor.matmul(
                                p[:], wv_sb[:, e * 128:(e + 1) * 128],
                                xts[:, e * 512:(e + 1) * 512],
                                start=(e == 0), stop=(e == 7))
                        if hi == 8:
                            vt = vtpool.tile([128, 512], BF16, tag="vt")
                            nc.vector.tensor_copy(vt[:], p[:])
                            state['vt'] = vt
                    return fn

                def vtr(k):
                    def fn():
                        pt = pm_pt[k % 2]
                        nc.tensor.transpose(pt, state['vt'][:, k * 128:(k + 1) * 128],
                                            identb[:])
                        nc.vector.tensor_copy(
                            v_sb[b][:, (st0 + k) * 128:(st0 + k + 1) * 128], pt)
                    return fn

                th = []
                for lo in range(0, 8, 4):
                    th.append((512 * 4, qk('q', wq_sb, qt_sb, lo, lo + 4)))
                for lo in range(0, 8, 4):
                    th.append((512 * 4, qk('k', wk_sb, kt_sb, lo, lo + 4)))
                for lo in range(0, 8, 4):
                    th.append((512 * 4, vmm(lo, lo + 4)))
                for k in range(4):
                    th.append((128, vtr(k)))
                return th

            # ---------- outproj (as thunks) ----------
            def outproj_thunks(b, tcg, attnS, last=False):
                state = {}

                def ft_fn(ft):
                    def fn():
                        if ft == 0:
                            state['o'] = opool.tile([128, 8 * 512], BF16,
                                                    tag="ob",
                                                    name=f"osb_{b}_{tcg}")
                        po = pp.tile([128, 512], F32, tag="pp",
                                     name=f"po_{b}_{tcg}_{ft}")
                        nc.tensor.matmul(po[:],
                                         wo_sb[:, ft * 128:(ft + 1) * 128],
                                         attnS[:],
                                         start=True, stop=True)
                        o_sb = state['o']
                        sl = o_sb[:, ft * 512:(ft + 1) * 512]
                        if last and ft >= 6:
                            nc.scalar.activation(sl, po[:], AF.Copy)
                        else:
                            nc.vector.tensor_copy(sl, po[:])
                        if ft == 7:
                            # one batched store per tcg
                            c0 = b * T + tcg * 512
                            nc.sync.dma_start(
                                out[:, c0:c0 + 512].rearrange(
                                    "(f p) m -> p f m", f=8),
                                o_sb[:].rearrange("p (f m) -> p f m", f=8))
                    return fn
                return [(512, ft_fn(ft)) for ft in range(8)]

            # ---------- attention ----------
            def scores_exp(b, tcg, st):
                """scores(st) into a rotating sab buffer + exp on ScalarE.
                Returns the e_t tile."""
                c0 = b * T + tcg * 512
                s0 = b * T + st * 128
                sab = ps.tile([128, 1024], F32, tag="sab")
                nc.tensor.matmul(sab[:, 0:512],
                                 kt_sb[0:64, s0:s0 + 128],
                                 qt_sb[0:64, c0:c0 + 512],
                                 start=True, stop=True)
                nc.tensor.matmul(sab[:, 512:1024],
                                 kt_sb[64:128, s0:s0 + 128],
                                 qt_sb[64:128, c0:c0 + 512],
                                 start=True, stop=True)
                e_t = epool.tile([128, 1024], BF16, tag="et")
                nc.scalar.activation(e_t[:], sab[:], AF.Exp, scale=0.125)
                return e_t

            def pv_mms(b, st, e_t):
                """Transposed PV + denominator mms for s-tile st."""
                for tcq in range(4):
                    for hh in range(2):
                        g = tcq * 2 + hh
                        lhs = e_t[:, hh * 512 + tcq * 128:hh * 512 + (tcq + 1) * 128]
                        nc.tensor.matmul(
                            pv[:, g * 64:(g + 1) * 64], lhs,
                            v_sb[b][:, st * 128 + hh * 64:st * 128 + hh * 64 + 64],
                            start=(st == 0), stop=(st == ST - 1))
                        nc.tensor.matmul(
                            pm_den[:, g:g + 1], lhs, ones_bf[:],
                            start=(st == 0), stop=(st == ST - 1))

            def finish_tcg(b, tcg):
                """Normalize attn^T, transpose to [d, t], return attnS tile."""
                rec = rcpool.tile([128, 8], F32, tag="rec")
                with nc.allow_low_precision(reason="softmax denom recip"):
                    nc.vector.reciprocal(rec[:], pm_den[:])
                attnS = aspool.tile([128, 512], BF16, tag="attnS",
                                    name=f"attnS_{b}_{tcg}")
                asr = {}
                for tcq in range(4):
                    asr[tcq] = asrpool.tile([128, 128], BF16, tag="asr",
                                            name=f"asr_{b}_{tcg}_{tcq}")
                    for hh in range(2):
                        g = tcq * 2 + hh
                        nc.vector.tensor_scalar_mul(
                            asr[tcq][:, hh * 64:(hh + 1) * 64],
                            pv[:, g * 64:(g + 1) * 64],
                            rec[:, g:g + 1])

                def tr_fn(tcq):
                    def fn():
                        t = pm_tr[tcq % 2]
                        nc.tensor.transpose(t, asr[tcq][:], identb[:])
                        nc.vector.tensor_copy(
                            attnS[:, tcq * 128:(tcq + 1) * 128], t)
                    return fn
                th = [(128, tr_fn(tcq)) for tcq in range(4)]
                return attnS, th

            # ---------- emission ----------
            load_weights()
            nc.scalar.activation(warm[:], identb[0:1, :].bitcast(F32),
                                 AF.Exp, scale=0.0)
            nc.vector.memset(ones_bf[:], 1.0)

            # batch-0 projection runs in the open (PE otherwise idle).
            # DMA order: weights, chunk 0-1, rope tables, chunk 2-3 — so the
            # first matmuls and the first rope are never DMA-starved.
            chunk_order = [(h, mc) for h in range(8) for mc in range(2)]
            xts_pending = {}
            for (h, mc) in chunk_order[:2]:
                xts_pending[(h, mc)] = load_x_half(h, mc)
            load_tables()
            for (h, mc) in chunk_order[2:4]:
                xts_pending[(h, mc)] = load_x_half(h, mc)
            for ci, (h, mc) in enumerate(chunk_order[:4]):
                if ci + 2 < len(chunk_order) and ci >= 2:
                    nh, nmc = chunk_order[ci + 2]
                    xts_pending[(nh, nmc)] = load_x_half(nh, nmc)
                for _, fn in proj_thunks(h, mc, xts_pending.pop((h, mc))):
                    fn()

            # enqueue helper: chunk DMAs issued two chunks ahead
            next_dma = [6]

            def enqueue_proj(ci):
                h, mc = chunk_order[ci]
                if (h, mc) not in xts_pending:
                    xts_pending[(h, mc)] = load_x_half(h, mc)
                xts = xts_pending.pop((h, mc))
                th = proj_thunks(h, mc, xts)

                def prefetch():
                    if next_dma[0] < len(chunk_order):
                        nh, nmc = chunk_order[next_dma[0]]
                        xts_pending[(nh, nmc)] = load_x_half(nh, nmc)
                        next_dma[0] += 1
                w0, f0 = th[0]

                def first():
                    prefetch()
                    f0()
                slow_q.append((w0, first))
                slow_q.extend(th[1:])

            # Per-window filler budget (PE cycles). Chosen so the slow queue
            # drains each batch's projections within the preceding batch's
            # attention phase even after the fast queue takes its share.
            FILL_W = 1000.0

            prev = None          # (b, tcg, attnS)
            for b in range(B):
                if b + 1 < B:
                    for ci in range(4 * (b + 1), 4 * (b + 2)):
                        enqueue_proj(ci)
                for tcg in range(4):
                    pend = None
                    for st in range(ST):
                        e_t = scores_exp(b, tcg, st)
                        if pend is not None:
                            pv_mms(b, pend[0], pend[1])
                        pend = (st, e_t)
                        emit_filler(FILL_W + (520 if st == 0 else 0))
                    pv_mms(b, pend[0], pend[1])
                    attnS, tr_th = finish_tcg(b, tcg)
                    fast_q.extend(tr_th)
                    if prev is not None:
                        # slow queue: outproj defers into the proj-starved
                        # final phase (attnS tiles stay alive meanwhile)
                        slow_q.extend(outproj_thunks(prev[0], prev[1], prev[2]))
                    prev = (b, tcg, attnS)
            drain_filler()
            for _, fn in outproj_thunks(prev[0], prev[1], prev[2], last=True):
                fn()

    nc.compile()
    return nc


def _host_prep(query, Wqkv, Wout):
    import ml_dtypes

    q32 = np.asarray(query, dtype=np.float32)
    # [T, B, E] -> [E, B, T] -> [E, B*T]  (column = b*T + t)
    xt = np.ascontiguousarray(q32.transpose(2, 1, 0).reshape(EMBED, M))

    # rope tables, fp16-rounded like the reference
    theta = np.power(ROPE_BASE,
                     -np.arange(0, HEAD_DIM, 2, dtype=np.float32) / HEAD_DIM)
    m_th = np.arange(T, dtype=np.float32)[:, None] * theta[None, :]
    m_th = np.concatenate([m_th, m_th], axis=-1)          # [T, 64]
    cos = np.cos(m_th).astype(np.float16).astype(np.float32)
    sin = np.sin(m_th).astype(np.float16).astype(np.float32)
    cosT = cos.T                                          # [64, T]
    sin_eff = sin.T.copy()
    sin_eff[0:32] = -sin_eff[0:32]
    cos128 = np.ascontiguousarray(np.concatenate([cosT, cosT], axis=0))
    sin128 = np.ascontiguousarray(np.concatenate([sin_eff, sin_eff], axis=0))

    # rotate-half permutation as a stationary matrix: prs = perm^T @ pr,
    # prs[i] = pr[perm(i)] with perm swapping 32-blocks within each 64-half
    perm = np.zeros((128, 128), dtype=np.float32)
    for i in range(128):
        j = (i // 64) * 64 + (i + 32) % 64
        perm[j, i] = 1.0
    perm = perm.astype(ml_dtypes.bfloat16)

    W = np.asarray(Wqkv, dtype=np.float32)
    Wo = np.asarray(Wout, dtype=np.float32)
    in_maps = []
    for c in range(NCORES):
        sl = slice(c * 128, (c + 1) * 128)
        in_maps.append({
            "xt": xt.astype(ml_dtypes.bfloat16),
            "wq": np.ascontiguousarray(W[sl, :].T).astype(ml_dtypes.bfloat16),
            "wk": np.ascontiguousarray(W[EMBED:][sl, :].T).astype(
                ml_dtypes.bfloat16),
            "wv": np.ascontiguousarray(W[2 * EMBED:][sl, :].T).astype(
                ml_dtypes.bfloat16),
            "wo": np.ascontiguousarray(Wo[:, sl].T).astype(ml_dtypes.bfloat16),
            "cosd": cos128,
            "sind": sin128,
            "identd": np.eye(128, dtype=np.float32).astype(ml_dtypes.bfloat16),
            "permd": perm,
        })
    return in_maps


def kernel(query, Wqkv, Wout):
    from concourse.bass_utils import run_bass_kernel_spmd

    nc = _build_program()
    in_maps = _host_prep(query, Wqkv, Wout)
    res = run_bass_kernel_spmd(nc, in_maps, core_ids=list(range(NCORES)))
    acc = np.zeros((EMBED, M), dtype=np.float32)
    for r in res.results:
        acc += np.asarray(r["out"], dtype=np.float32)
    # out^T [E, b*T+t] -> [B, T, E] -> [T, B, E]
    full = acc.T.reshape(B, T, EMBED).transpose(1, 0, 2)
    return np.ascontiguousarray(full)
